# revision 32
# baseline (speedup 1.0000x reference)
"""2-layer GAT + MLP head on 8 TRN2 NeuronCores.

Strategy (dst-sharded, layer-0 aggregation folded into x-space):
- Nodes padded to NP=20480; each core owns a contiguous 2560-dst shard.
  Edges (incl. self-loops, PyG mean-fill edge attr) sorted by dst,
  grouped into 128-dst tiles, padded per tile-slot to a chunk count K_t
  shared by all cores (SPMD: one program).
- Layer 0 aggregates RAW x per head (W0 applied after the softmax
  average, per dst tile): the gather table is x padded to 128 bf16 cols
  [x(64) | 1 | 0*3 | asrc0 f32-bits(8) | 0...]; the ones column makes
  the softmax denominator fall out of the same one-hot matmul.
  asrc0/adst0 are host-folded (O(N*F_IN*H) prep).
- Layer 1 gathers 1280B rows of the H1 table [h1 bf16(512) | asrc1,
  adst1 f32-bits(16) | pad]; H1 is assembled by NAG group-wise
  AllGathers (Shared scratchpad) that overlap layer-0 finalize.
- Per 128-edge chunk: p = exp(leakyrelu(asrc+adst+ae)) batched per
  super-chunk; adst expanded per edge via a one-hot-transpose matmul
  (adst rows live in SBUF: host const for L0, fin0-written for L1);
  out[dst] accumulated via one-hot matmuls in PSUM.
- ohb/oht one-hot blocks stream from HBM as one packed bf16 tensor.
"""

import numpy as np
import ml_dtypes

import concourse.bacc as bacc
import concourse.bass as bass
import concourse.mybir as mybir
import concourse.tile as tile
from concourse.bass_utils import run_bass_kernel_spmd

F32 = mybir.dt.float32
BF16 = mybir.dt.bfloat16
FP8 = mybir.dt.float8e4
I16 = mybir.dt.int16
AF = mybir.ActivationFunctionType
OP = mybir.AluOpType

NCORES = 8
SCC = 8    # chunks (of 128 edges) per gather super-chunk
GSPLIT = 4  # gather instructions per super-chunk
B0 = 68    # layer-0 per-head block: x(64) | ones | pad(3)


def _bcast_mid(ap_sl, reps):
    """[128, F] -> [128, reps, F] broadcasting the middle axis."""
    return bass.AP(ap_sl.tensor, ap_sl.offset,
                   [list(ap_sl.ap[0]), [0, reps], list(ap_sl.ap[-1])])


def _bcast_last(ap_sl, reps):
    """[128, M] -> [128, M, reps] broadcasting the last axis."""
    return bass.AP(ap_sl.tensor, ap_sl.offset,
                   [list(ap_sl.ap[0]), list(ap_sl.ap[-1]), [0, reps]])


def _build_program(NP, F_IN, HC, H, C, NT, K_t, FTS, GS, hi_q,
                   seq, tile_slots,
                   use_b0, use_b1, use_l0b, use_l1b):
    NCHUNK = int(sum(K_t))
    KMAX = max(K_t)
    E_pad = NCHUNK * 128
    SW = E_pad // 16
    TW = HC + 128          # L1 table row: h | asrc,adst (f32 bits) | pad
    KB = HC // 128
    NAG = len(GS)          # allgather groups (tile counts, uneven ok)
    RG = [0]
    for gsz in GS:
        RG.append(RG[-1] + gsz)
    g_of_t = []
    for gi, gsz in enumerate(GS):
        g_of_t += [gi] * gsz

    nc = bacc.Bacc(dynamic_dma_scratch_size=65536, num_swdge_queues=4)
    P = nc.declare_dram_parameter

    xgs = P("xgs", [128, NCHUNK * 80], BF16, isOutput=False)
    r1h = P("r1h", [HC, HC], BF16, isOutput=False)
    r1a = P("r1a", [HC, 8], BF16, isOutput=False)
    r2 = P("r2", [HC, FTS], BF16, isOutput=False)
    r3 = P("r3", [FTS, 1], BF16, isOutput=False)
    w0h = P("w0h", [64, HC], BF16, isOutput=False)
    b1t = P("b1t", [128, HC], F32, isOutput=False)
    l0bt = P("l0bt", [128, FTS], F32, isOutput=False)
    l1bt = P("l1bt", [128, 1], F32, isOutput=False)
    ident = P("ident", [128, 128], F32, isOutput=False)
    identb = P("identb", [128, 128], BF16, isOutput=False)
    srcw = P("srcw", [128, SW], I16, isOutput=False)
    ohb = P("ohb", [128, NCHUNK * 128], FP8, isOutput=False)
    oht = P("oht", [128, NCHUNK * 128], FP8, isOutput=False)
    ae1 = P("ae1", [128, NCHUNK, 4], BF16, isOutput=False)
    adt0 = P("adt0", [128, NT * 4], BF16, isOutput=False)
    outp = P("out", [NT * 128, 1], F32, isOutput=True)

    # chunk q -> owning dst tile (pair-interleaved schedule)
    t_of_q = [t for (t, k) in seq]
    SL0 = [tile_slots[t][0] for t in range(NT)]
    SSTR = [(tile_slots[t][1] - tile_slots[t][0]) if K_t[t] > 1 else 1
            for t in range(NT)]
    for t in range(NT):
        d = np.diff(tile_slots[t])
        assert len(d) == 0 or (d == d[0]).all(), "irregular slot stride"


    with tile.TileContext(nc) as tc:
        with (
            tc.tile_pool(name="const", bufs=1) as const,
            tc.tile_pool(name="stage", bufs=3) as stage,
            tc.tile_pool(name="work", bufs=3) as work,
            tc.tile_pool(name="gpp", bufs=6) as gpp,
            tc.tile_pool(name="tp", bufs=6) as tp,
            tc.tile_pool(name="psacc", bufs=2, space="PSUM") as psacc,
            tc.tile_pool(name="psfin", bufs=1, space="PSUM") as psfin,
            tc.tile_pool(name="pss", bufs=2, space="PSUM") as pss,
            tc.tile_pool(name="pstr", bufs=2, space="PSUM") as pstr,
            tc.tile_pool(name="pspd", bufs=1, space="PSUM") as pspd,
            tc.tile_pool(name="dram", bufs=1, space="DRAM") as dram,
        ):
            AGW = TW            # full-width AG rows (direct into H1)
            H1 = dram.tile([NP, TW], BF16, tag="H1")
            H1g = [dram.tile([GS[g] * 128, AGW], BF16, tag=f"H1g{g}",
                             name=f"H1g{g}")
                   for g in range(NAG)]

            _cn = [0]

            def cload(ap_in, shape, dt=F32, tag=None):
                _cn[0] += 1
                cname = tag or f"c{_cn[0]}"
                t = const.tile(shape, dt, tag=cname, name=f"{cname}_{_cn[0]}")
                nc.sync.dma_start(out=t[:], in_=ap_in)
                return t

            r1h_s = [cload(r1h[k * 128:(k + 1) * 128, :], [128, HC], BF16)
                     for k in range(KB)]
            r1a_s = [cload(r1a[k * 128:(k + 1) * 128, :], [128, 8], BF16)
                     for k in range(KB)]
            r2_s = [cload(r2[k * 128:(k + 1) * 128, :], [128, FTS], BF16)
                    for k in range(KB)]
            r3_s = cload(r3[:, :], [FTS, 1], BF16)
            w0h_s = cload(w0h[:, :], [64, HC], BF16)
            l0b_s = cload(l0bt[:, :], [128, FTS]) if use_l0b else None
            l1b_s = cload(l1bt[:, :], [128, 1]) if use_l1b else None
            b1_s = cload(b1t[:, :], [128, HC]) if use_b1 else None
            id_s = cload(ident[:, :], [128, 128])
            idb_s = cload(identb[:, :], [128, 128], BF16)
            srcw_s = cload(srcw[:, :], [128, SW], I16)
            ae1_s = cload(ae1[:, :, :], [128, NCHUNK, 4], BF16, tag="ae1")
            adt0_s = cload(adt0[:, :], [128, NT * 4], BF16, tag="adt0")
            # L1 adst rows, written by fin0 tile by tile
            adt1_s = const.tile([128, NT * 4], BF16, tag="adt1", name="adt1")
            # adst expanded per edge-slot, precomputed out of the hot loop
            pd0_all = const.tile([128, NCHUNK, 4], BF16, tag="pd0", name="pd0")
            pd1_all = const.tile([128, NCHUNK, 4], BF16, tag="pd1", name="pd1")

            # chunk ranges per tile
            qstart = [0]
            for t in range(NT):
                qstart.append(qstart[-1] + K_t[t])

            def pead_tile(t, adt_s, pd_all, ae_s=None):
                """adst-expand all chunks of tile t via oht matmuls."""
                kt = K_t[t]
                q0 = qstart[t]
                ot = stage.tile([128, KMAX * 128], FP8, tag="ohtp",
                                name="ohtp", bufs=2)
                nc.sync.dma_start(out=ot[:, 0:kt * 128],
                                  in_=oht[:, q0 * 128:(q0 + kt) * 128])
                pdp = pspd.tile([128, KMAX * 4], F32, tag="pd", name="pd")
                for k in range(kt):
                    nc.tensor.matmul(
                        pdp[:, k * 4:(k + 1) * 4],
                        ot[:, k * 128:(k + 1) * 128],
                        adt_s[:, t * 4:(t + 1) * 4],
                        start=True, stop=True)
                pdv = pdp[:, 0:kt * 4].rearrange("x (a b) -> x a b", b=4)

                def strided(tile_, kt_):
                    sl = tile_[:, SL0[t]:SL0[t] + kt_, :]
                    return bass.AP(sl.tensor, sl.offset,
                                   [list(sl.ap[0]),
                                    [sl.ap[1][0] * SSTR[t], kt_],
                                    list(sl.ap[2])])

                if ae_s is None:
                    nc.vector.tensor_copy(strided(pd_all, kt), pdv)
                else:
                    # fold the per-edge ae term in here (saves an add in
                    # the hot per-super chain)
                    nc.vector.tensor_add(strided(pd_all, kt), pdv,
                                         strided(ae_s, kt))

            def agg_layer(lname, elem, pd_all, finalize, pre_tile=None):
                def ensure_super(s):
                    cnt = min(SCC * 128, E_pad - s * SCC * 128)
                    nch = cnt // 128
                    g = stage.tile([128, SCC, elem], BF16,
                                   tag=f"g{lname}", name=f"g{lname}",
                                   bufs=(4 if lname == "l0" else 5))
                    if lname == "l0":
                        # host pre-gathered edge stream, plain DMA
                        nc.sync.dma_start(
                            out=g[:, 0:nch, :],
                            in_=xgs[:, s * SCC * 80:(s * SCC + nch) * 80])
                    else:
                        c0 = s * SCC * 8
                        nq = min(GSPLIT, nch)
                        base = 0
                        for qi in range(nq):
                            take = (nch - base + (nq - qi) - 1) // (nq - qi)
                            # rows are src-sorted per tile: this split only
                            # reads H1[:hi], so its gather unlocks as soon
                            # as the AllGather groups covering those rows
                            # have landed (overlaps the AG tail)
                            hi = int(max(hi_q[s * SCC + base:
                                             s * SCC + base + take]))
                            nc.gpsimd.dma_gather(
                                g[:, base:base + take, :], H1[0:hi, :],
                                srcw_s[:, c0 + base * 8:
                                       c0 + (base + take) * 8],
                                take * 128, take * 128, elem,
                                single_packet=True,
                                queue_num=(s * nq + qi) % 4)
                            base += take
                    oh = stage.tile([128, SCC, 128], FP8, tag="oh",
                                    name="oh", bufs=6)
                    nc.sync.dma_start(
                        out=oh[:, 0:nch, :],
                        in_=ohb[:, s * SCC * 128:(s * SCC + nch) * 128])
                    if lname == "l0":
                        # asrc0 + ae0 pre-folded on host into the stream
                        t0v = g[:, 0:nch, 68:76].bitcast(F32)
                    else:
                        # ae1 folded into pd_all at pead time
                        t0v = g[:, 0:nch, HC:HC + 8].bitcast(F32)
                    t1 = work.tile([128, SCC, 4], F32, tag="t1", bufs=6)
                    nc.vector.tensor_add(
                        t1[:, 0:nch, :], t0v,
                        pd_all[:, s * SCC:s * SCC + nch, :])
                    t3 = work.tile([128, SCC, 4], F32, tag="t3", bufs=6)
                    nc.scalar.activation(t3[:, 0:nch, :], t1[:, 0:nch, :],
                                         AF.Prelu, alpha=0.2)
                    pb = work.tile([128, SCC, 4], BF16, tag="pb", bufs=6)
                    nc.scalar.activation(pb[:, 0:nch, :], t3[:, 0:nch, :],
                                         AF.Exp)
                    # p-weighted gather rows for the whole super, one DVE op
                    BW = B0 if lname == "l0" else C
                    gps = gpp.tile([128, SCC, 4, BW], BF16,
                                   tag=f"gps{lname}", name=f"gps{lname}",
                                   bufs=3)
                    # two halves so downstream matmuls start sooner
                    h1n = min(nch, SCC // 2)
                    for (a, b) in ((0, h1n), (h1n, nch)):
                        if b <= a:
                            continue
                        if lname == "l0":
                            sl = g[:, a:b, 0:BW]
                            i0 = bass.AP(sl.tensor, sl.offset,
                                         [list(sl.ap[0]), list(sl.ap[1]),
                                          [0, 4], list(sl.ap[2])])
                        else:
                            i0 = g[:, a:b, 0:HC].rearrange(
                                "x a (h c) -> x a h c", h=H)
                        psl = pb[:, a:b, :]
                        i1 = bass.AP(psl.tensor, psl.offset,
                                     [list(psl.ap[0]), list(psl.ap[1]),
                                      list(psl.ap[2]), [0, BW]])
                        nc.vector.tensor_mul(gps[:, a:b, :, :], i0, i1)
                    return gps, oh, pb

                gps = oh = pb = None
                BW = B0 if lname == "l0" else C
                ps_open = {}
                pssT = [None]
                for q, (t, k) in enumerate(seq):
                    s, j = divmod(q, SCC)
                    if j == 0:
                        if pre_tile is not None:
                            pre_tile(max(t_of_q[s * SCC:
                                               min((s + 1) * SCC, NCHUNK)]))
                        gps, oh, pb = ensure_super(s)
                    if k == 0:
                        if lname == "l0":
                            ps_open[t] = (psacc.tile([128, 4 * B0], F32,
                                                     tag="ps",
                                                     name="ps0"), None)
                        else:
                            ps_open[t] = (psacc.tile([128, HC], F32,
                                                     tag="ps",
                                                     name="ps1"),
                                          pss.tile([128, 8], F32, tag="pss",
                                                   name="pss1"))
                    ps_o, ps_s = ps_open[t]
                    first, last = (k == 0), (k == K_t[t] - 1)
                    oh_j = oh[:, j, :]
                    rhs = gps[:, j, :, :].rearrange(
                        "x h c -> x (h c)")
                    nc.tensor.matmul(ps_o[:], oh_j, rhs,
                                     start=first, stop=last)
                    if lname != "l0":
                        nc.tensor.matmul(ps_s[:, 0:4], oh_j, pb[:, j, :],
                                         start=first, stop=last)
                    if last:
                        finalize(t, ps_o, ps_s)
                        del ps_open[t]

            # ---- layer-0 finalize: normalize in x-space, apply W0, relu,
            #      layer-1 linear, H1 group AllGather ----
            def fin0(t, ps_o, ps_s):
                po = ps_o[:].rearrange("x (h c) -> x h c", h=H)
                sp = work.tile([128, 4], F32, tag="sp")
                nc.vector.tensor_scalar_add(sp[:], po[:, :, 64], 1e-16)
                rc = work.tile([128, 4], F32, tag="rc")
                nc.vector.reciprocal(rc[:], sp[:])
                ax = work.tile([128, 4, 64], BF16, tag="ax")
                nc.vector.tensor_mul(ax[:], po[:, :, 0:64],
                                     _bcast_last(rc[:], 64))
                a0k = []
                for h in range(H):
                    ptb = pstr.tile([64, 128], BF16, tag="pt")
                    nc.tensor.transpose(ptb[:], ax[:, h, :], idb_s[:])
                    ak = tp.tile([64, 128], BF16, tag="axT", name=f"axT{h}")
                    nc.vector.tensor_copy(ak[:], ptb[:])
                    a0k.append(ak)
                out0 = psfin.tile([128, HC], F32, tag="pf")
                for h in range(H):
                    nc.tensor.matmul(out0[:, h * 128:(h + 1) * 128],
                                     a0k[h][:], w0h_s[:, h * 128:(h + 1) * 128],
                                     start=True, stop=True)
                ar = work.tile([128, HC], BF16, tag="ar", bufs=2)
                nc.vector.tensor_scalar_max(ar[:], out0[:], 0.0)
                a1 = []
                for kk in range(KB):
                    pt = pstr.tile([128, 128], BF16, tag="pt")
                    nc.tensor.transpose(pt[:], ar[:, kk * 128:(kk + 1) * 128],
                                        idb_s[:])
                    ak = tp.tile([128, 128], BF16, tag="a1T", name=f"a0k{kk}")
                    nc.vector.tensor_copy(ak[:], pt[:])
                    a1.append(ak)
                ph1 = psfin.tile([128, HC], F32, tag="pf")
                pa1 = pss.tile([128, 8], F32, tag="pss")
                for kk in range(KB):
                    first, last = (kk == 0), (kk == KB - 1)
                    nc.tensor.matmul(ph1[:], a1[kk][:], r1h_s[kk][:],
                                     start=first, stop=last)
                    nc.tensor.matmul(pa1[:], a1[kk][:], r1a_s[kk][:],
                                     start=first, stop=last)
                # stash adst1 (bf16), then expand it for tile t's edge slots
                nc.vector.tensor_copy(adt1_s[:, t * 4:(t + 1) * 4],
                                      pa1[:, 4:8])
                pead_tile(t, adt1_s, pd1_all, ae_s=ae1_s)
                st = stage.tile([128, AGW], BF16, tag="hrow")
                if t % 2 == 0:
                    nc.vector.tensor_copy(st[:, 0:HC], ph1[:])
                else:
                    nc.scalar.activation(st[:, 0:HC], ph1[:], AF.Copy)
                nc.scalar.activation(st[:, HC:HC + 16].bitcast(F32),
                                     pa1[:], AF.Copy)
                gidx = g_of_t[t]
                loc = t - RG[gidx]
                nc.sync.dma_start(out=H1g[gidx][loc * 128:(loc + 1) * 128, :],
                                  in_=st[:])
                if loc == GS[gidx] - 1:
                    r0 = RG[gidx] * NCORES * 128
                    r1 = RG[gidx + 1] * NCORES * 128
                    nc.gpsimd.collective_compute(
                        "AllGather", OP.bypass,
                        replica_groups=[list(range(NCORES))],
                        ins=[H1g[gidx].opt()],
                        outs=[H1[r0:r1, :].opt()],
                    )

            # ---- layer-1 finalize: normalize + relu + MLP head ----
            def fin1(t, ps_o, ps_s):
                sp = work.tile([128, 4], F32, tag="sp")
                nc.vector.tensor_scalar_add(sp[:], ps_s[:, 0:4], 1e-16)
                rc = work.tile([128, 4], F32, tag="rc")
                nc.vector.reciprocal(rc[:], sp[:])
                ao = work.tile([128, HC], F32, tag="ao", bufs=2)
                nc.vector.tensor_mul(
                    ao[:].rearrange("x (h c) -> x h c", h=H),
                    ps_o[:].rearrange("x (h c) -> x h c", h=H),
                    _bcast_last(rc[:], C))
                if use_b1:
                    ab = work.tile([128, HC], F32, tag="ao", bufs=2)
                    nc.vector.tensor_add(ab[:], ao[:], b1_s[:])
                    ao2 = ab
                else:
                    ao2 = ao
                ar = work.tile([128, HC], BF16, tag="ar1", bufs=2)
                nc.vector.tensor_scalar_max(ar[:], ao2[:], 0.0)
                h2p = psfin.tile([128, FTS], F32, tag="pf")
                for kk in range(KB):
                    pt = pstr.tile([128, 128], BF16, tag="pt")
                    nc.tensor.transpose(pt[:], ar[:, kk * 128:(kk + 1) * 128],
                                        idb_s[:])
                    a1k = tp.tile([128, 128], BF16, tag="a1T32")
                    nc.scalar.activation(a1k[:], pt[:], AF.Copy)
                    nc.tensor.matmul(h2p[:], a1k[:], r2_s[kk][:],
                                     start=(kk == 0), stop=(kk == KB - 1))
                if use_l0b:
                    h2b = work.tile([128, FTS], F32, tag="h2b")
                    nc.vector.tensor_add(h2b[:], h2p[:], l0b_s[:])
                else:
                    h2b = h2p
                h2r = work.tile([128, FTS], BF16, tag="h2r")
                nc.vector.tensor_scalar_max(h2r[:], h2b[:], 0.0)
                pt2 = pstr.tile([128, 128], BF16, tag="pt")
                nc.tensor.transpose(pt2[:], h2r[:], idb_s[:])
                h2T = tp.tile([128, 128], BF16, tag="a1T32")
                nc.scalar.activation(h2T[:], pt2[:], AF.Copy)
                po = pss.tile([128, 8], F32, tag="pss")
                nc.tensor.matmul(po[:, 0:1], h2T[:], r3_s[:],
                                 start=True, stop=True)
                ob = work.tile([128, 1], F32, tag="ob")
                if use_l1b:
                    nc.vector.tensor_add(ob[:], po[:, 0:1], l1b_s[:])
                else:
                    nc.vector.tensor_copy(ob[:], po[:, 0:1])
                nc.sync.dma_start(out=outp[t * 128:(t + 1) * 128, :],
                                  in_=ob[:])

            # pead for layer 0 is pipelined into the aggregation loop:
            # before tile t's chunks run, peads are emitted for every
            # tile any super ensured during t can touch
            pead_next = [0]

            def pre0(tl):
                while pead_next[0] <= tl:
                    pead_tile(pead_next[0], adt0_s, pd0_all)
                    pead_next[0] += 1

            nc._state.push_named_scope("phaseB")
            agg_layer("l0", 80, pd0_all, fin0, pre_tile=pre0)
            nc._state.pop_named_scope("phaseB")
            nc._state.push_named_scope("phaseD")
            agg_layer("l1", TW, pd1_all, fin1)
            nc._state.pop_named_scope("phaseD")

    nc.finalize()
    return nc


def _wrap_idx(v, E_pad):
    blk = np.zeros((16, E_pad // 16), np.int16)
    ar = np.arange(E_pad)
    blk[ar % 16, ar // 16] = v.astype(np.int16)
    return np.tile(blk, (8, 1))


def kernel(x, edge_index, edge_weights,
           W0, as0, ad0, We0, ae0, b0,
           W1, as1, ad1, We1, ae1, b1,
           L0W, L0b, L1W, L1b):
    x = np.asarray(x, np.float32)
    N, F_IN = x.shape
    HC = W0.shape[0]
    H, C = np.asarray(as0).shape
    FTS = np.asarray(L0W).shape[0]

    NT = -(-N // (128 * NCORES))
    SHARD = NT * 128
    NP = SHARD * NCORES
    # allgather groups (tile counts): sized so each group's transfer keeps
    # pace with layer-0 tile production; 1-tile tail minimizes the
    # phase-boundary stall
    if NT == 20:
        GS = [4, 4, 4, 4, 2, 2]   # pair-aligned (tiles finalize in pairs)
    else:
        GS = [NT]
    RG = np.zeros(len(GS) + 1, np.int64)
    RG[1:] = np.cumsum(GS)
    g_of_t = np.repeat(np.arange(len(GS)), GS)

    # ---- edges ----
    ew_in = np.asarray(edge_weights, np.float32)
    src = np.concatenate([np.asarray(edge_index[0]), np.arange(N)])
    dst = np.concatenate([np.asarray(edge_index[1]), np.arange(N)])
    ew = np.concatenate([ew_in, np.full(N, ew_in.mean(), np.float32)])

    # ---- degree-balanced node -> (core, tile, slot) assignment ----
    # LPT-pack nodes into NTG bins of 128 by in-degree, then deal bins to
    # (tile, core) rank-major so per-tile maxima (=> K_t padding) equalize
    NTG = NP // 128
    nodes = np.arange(NP)
    deg = np.bincount(dst, minlength=NP)
    import heapq
    heap = [(0, b) for b in range(NTG)]
    heapq.heapify(heap)
    bincnt = np.zeros(NTG, np.int64)
    binsum = np.zeros(NTG, np.int64)
    bin_of_n = np.empty(NP, np.int64)
    for n in np.argsort(-deg, kind="stable"):
        while True:
            s, b = heapq.heappop(heap)
            if bincnt[b] < 128:
                break
        bin_of_n[n] = b
        binsum[b] += deg[n]
        bincnt[b] += 1
        if bincnt[b] < 128:
            heapq.heappush(heap, (int(binsum[b]), b))
    brank = np.argsort(-binsum, kind="stable")
    tile_of_bin = np.empty(NTG, np.int64)
    core_of_bin = np.empty(NTG, np.int64)
    tile_of_bin[brank] = np.arange(NTG) // NCORES
    core_of_bin[brank] = np.arange(NTG) % NCORES
    n_tile = tile_of_bin[bin_of_n]
    n_core = core_of_bin[bin_of_n]
    gt_of_n = n_core * NT + n_tile               # node -> global tile
    ord2 = np.argsort(gt_of_n, kind="stable")
    n_slot = np.empty(NP, np.int64)
    n_slot[ord2] = np.arange(NP) % 128
    node_of = np.empty(NP, np.int64)             # (gtile*128+slot) -> node
    node_of[gt_of_n * 128 + n_slot] = nodes

    # table-row permutation (group-major) so group AllGathers land contiguous
    gg = g_of_t[n_tile]
    t_of_n = (RG[gg] * NCORES * 128 + n_core * np.asarray(GS)[gg] * 128
              + (n_tile - RG[gg]) * 128 + n_slot)    # node -> table row

    order = np.argsort(gt_of_n[dst], kind="stable")
    src_s, dst_s, ew_s = src[order], dst[order], ew[order]

    tile_of = gt_of_n[dst_s]
    tcounts = np.bincount(tile_of, minlength=NTG)
    tstart = np.concatenate([[0], np.cumsum(tcounts)])

    K_t = [max(1, int(max(-(-tcounts[i * NT + t] // 128)
                          for i in range(NCORES))))
           for t in range(NT)]
    # equalize within pairs so the interleaved schedule has regular stride
    for j2 in range(0, NT - 1, 2):
        m = max(K_t[j2], K_t[j2 + 1])
        K_t[j2] = K_t[j2 + 1] = m
    NCHUNK = int(sum(K_t))
    E_pad = NCHUNK * 128
    qstart_h = np.concatenate([[0], np.cumsum(K_t)]).astype(np.int64)

    # pair-interleaved chunk schedule: tiles 2j/2j+1 alternate chunks so
    # src-sorted gathers unlock group-by-group across two tiles at once
    seq = []
    tile_slots = [[] for _ in range(NT)]
    for j2 in range(0, NT, 2):
        ta, tb = j2, min(j2 + 1, NT - 1)
        for k in range(K_t[ta]):
            tile_slots[ta].append(len(seq))
            seq.append((ta, k))
            if tb != ta and k < K_t[tb]:
                tile_slots[tb].append(len(seq))
                seq.append((tb, k))

    # ---- weight folding (host, O(weights) + O(N*F_IN*H)) ----
    as0 = np.asarray(as0, np.float32)
    ad0 = np.asarray(ad0, np.float32)
    ae0w = np.asarray(ae0, np.float32)
    as1 = np.asarray(as1, np.float32)
    ad1 = np.asarray(ad1, np.float32)
    ae1w = np.asarray(ae1, np.float32)
    W0 = np.asarray(W0, np.float32)
    W1 = np.asarray(W1, np.float32)
    We0 = np.asarray(We0, np.float32)
    We1 = np.asarray(We1, np.float32)

    k0 = (We0.reshape(H, C) * ae0w).sum(1).astype(np.float32)
    k1 = (We1.reshape(H, C) * ae1w).sum(1).astype(np.float32)

    def fold(W, a):
        blk = np.zeros((HC, H), np.float32)
        for h in range(H):
            blk[h * C:(h + 1) * C, h] = a[h]
        return (W.T @ blk).astype(np.float32)

    bf = ml_dtypes.bfloat16
    r1h = W1.T.astype(bf)
    r1a = np.concatenate([fold(W1, as1), fold(W1, ad1)], 1).astype(bf)
    r2 = np.asarray(L0W, np.float32).T.astype(bf)
    r3 = np.asarray(L1W, np.float32).T.astype(bf)
    w0h = W0.T.astype(bf)           # [64, 512]; cols h*128.. = W0_h^T

    # per-node layer-0 attention terms (tiny host matmuls)
    asrc0 = (x @ fold(W0, as0)).astype(np.float32)   # [N, 4]
    adst0 = (x @ fold(W0, ad0)).astype(np.float32)   # [N, 4]

    xbf = x.astype(bf)                               # node-order x, bf16
    adsta = np.zeros((NP, 4), np.float32)
    adsta[:N] = adst0

    b1t = np.tile(np.asarray(b1, np.float32)[None, :], (128, 1))
    l0bt = np.tile(np.asarray(L0b, np.float32)[None, :], (128, 1))
    l1bt = np.tile(np.asarray(L1b, np.float32).reshape(1, 1), (128, 1))
    ident = np.eye(128, dtype=np.float32)
    identb = np.eye(128, dtype=np.float32).astype(bf)

    in_maps = []
    srcp_all = []
    for i in range(NCORES):
        srcp = np.zeros(E_pad, np.int64)
        srcn = np.zeros(E_pad, np.int64)         # node-id src (host gather)
        dlocp = np.full(E_pad, -1, np.int64)
        dlocp_tm = np.full(E_pad, -1, np.int64)  # tile-major (oht/pead)
        ewp = np.zeros(E_pad, np.float32)
        for t in range(NT):
            gt = i * NT + t
            cnt = int(tcounts[gt])
            sl = slice(tstart[gt], tstart[gt] + cnt)
            # order tile's edges by src table row: the L1 gather then
            # walks H1 monotonically (better HBM behavior, dups adjacent)
            so = np.argsort(t_of_n[src_s[sl]], kind="stable")
            tsp = t_of_n[src_s[sl]][so]
            tsn = src_s[sl][so]
            tdl = n_slot[dst_s[sl]][so]
            tew = ew_s[sl][so]
            o_tm = qstart_h[t] * 128
            dlocp_tm[o_tm:o_tm + cnt] = tdl
            for k in range(K_t[t]):
                lo = k * 128
                if lo >= cnt:
                    break
                hi2 = min(lo + 128, cnt)
                qd = tile_slots[t][k] * 128
                srcp[qd:qd + hi2 - lo] = tsp[lo:hi2]
                srcn[qd:qd + hi2 - lo] = tsn[lo:hi2]
                dlocp[qd:qd + hi2 - lo] = tdl[lo:hi2]
                ewp[qd:qd + hi2 - lo] = tew[lo:hi2]
        srcp_all.append(srcp.copy())
        ae1p = (ewp[:, None] * k1[None, :]).reshape(
            NCHUNK, 128, 4).transpose(1, 0, 2)
        # layer-0 edge stream: [x(64) | 1 | 0*3 | asrc0+ae0 f32 bits | pad]
        xg_u16 = np.zeros((E_pad, 80), np.uint16)
        xg_u16[:, 0:64] = xbf[srcn].view(np.uint16)
        xg_u16[:, 64] = np.array(1.0, bf).view(np.uint16)
        l0a = (asrc0[srcn] + ewp[:, None] * k0[None, :]).astype(np.float32)
        xg_u16[:, 68:76] = np.ascontiguousarray(l0a).view(np.uint16)
        xgs_np = np.ascontiguousarray(
            xg_u16.reshape(NCHUNK, 128, 80).transpose(1, 0, 2)
            .reshape(128, NCHUNK * 80)).view(bf)
        dl2 = dlocp.reshape(NCHUNK, 128)
        valid = dl2 >= 0
        qs, es = np.nonzero(valid)
        f8 = ml_dtypes.float8_e4m3fn
        # ohb[e, q, d]: partition = edge-slot e
        ohb_np = np.zeros((128, NCHUNK, 128), f8)
        ohb_np[es, qs, dl2[qs, es]] = 1.0
        ohb_np = np.ascontiguousarray(ohb_np.reshape(128, NCHUNK * 128))
        # oht[d, q, e]: partition = dst-slot d; TILE-MAJOR chunk order
        dl2t = dlocp_tm.reshape(NCHUNK, 128)
        qs2, es2 = np.nonzero(dl2t >= 0)
        oht_np = np.zeros((128, NCHUNK, 128), f8)
        oht_np[dl2t[qs2, es2], qs2, es2] = 1.0
        oht_np = np.ascontiguousarray(oht_np.reshape(128, NCHUNK * 128))
        # adst0 rows for this core's dst tiles
        adt0_np = np.zeros((128, NT * 4), np.float32)
        for t in range(NT):
            rows = node_of[(i * NT + t) * 128 + np.arange(128)]
            adt0_np[:, t * 4:(t + 1) * 4] = adsta[rows]
        in_maps.append({
            "xgs": xgs_np, "r1h": r1h, "r1a": r1a, "r2": r2, "r3": r3,
            "w0h": w0h, "b1t": b1t, "l0bt": l0bt, "l1bt": l1bt,
            "ident": ident, "identb": identb,
            "srcw": _wrap_idx(srcp, E_pad),
            "ohb": ohb_np, "oht": oht_np,
            "ae1": np.ascontiguousarray(ae1p).astype(bf),
            "adt0": adt0_np.astype(bf),
        })

    hi_q = (np.stack(srcp_all).reshape(NCORES, NCHUNK, 128)
            .max(axis=(0, 2)) + 1)
    nc = _build_program(NP, F_IN, HC, H, C, NT, K_t, FTS, GS, hi_q,
                        seq, tile_slots,
                        bool(np.any(b0)), bool(np.any(b1)),
                        bool(np.any(np.asarray(L0b))),
                        bool(np.any(np.asarray(L1b))))
    res = run_bass_kernel_spmd(nc, in_maps, list(range(NCORES)))
    cat = np.concatenate([res.results[i]["out"][:, 0] for i in range(NCORES)])
    out_full = np.empty(NP, np.float32)
    out_full[node_of] = cat
    return out_full[:N].astype(np.float32)



# revision 33
# speedup vs baseline: 1.0581x; 1.0581x over previous
"""2-layer GAT + MLP head on 8 TRN2 NeuronCores.

Strategy (dst-sharded, layer-0 aggregation folded into x-space):
- Nodes padded to NP=20480; each core owns a contiguous 2560-dst shard.
  Edges (incl. self-loops, PyG mean-fill edge attr) sorted by dst,
  grouped into 128-dst tiles, padded per tile-slot to a chunk count K_t
  shared by all cores (SPMD: one program).
- Layer 0 aggregates RAW x per head (W0 applied after the softmax
  average, per dst tile): the gather table is x padded to 128 bf16 cols
  [x(64) | 1 | 0*3 | asrc0 f32-bits(8) | 0...]; the ones column makes
  the softmax denominator fall out of the same one-hot matmul.
  asrc0/adst0 are host-folded (O(N*F_IN*H) prep).
- Layer 1 gathers 1280B rows of the H1 table [h1 bf16(512) | asrc1,
  adst1 f32-bits(16) | pad]; H1 is assembled by NAG group-wise
  AllGathers (Shared scratchpad) that overlap layer-0 finalize.
- Per 128-edge chunk: p = exp(leakyrelu(asrc+adst+ae)) batched per
  super-chunk; adst expanded per edge via a one-hot-transpose matmul
  (adst rows live in SBUF: host const for L0, fin0-written for L1);
  out[dst] accumulated via one-hot matmuls in PSUM.
- ohb/oht one-hot blocks stream from HBM as one packed bf16 tensor.
"""

import numpy as np
import ml_dtypes

import concourse.bacc as bacc
import concourse.bass as bass
import concourse.mybir as mybir
import concourse.tile as tile
from concourse.bass_utils import run_bass_kernel_spmd

F32 = mybir.dt.float32
BF16 = mybir.dt.bfloat16
FP8 = mybir.dt.float8e4
I16 = mybir.dt.int16
AF = mybir.ActivationFunctionType
OP = mybir.AluOpType

NCORES = 8
SCC = 8    # chunks (of 128 edges) per gather super-chunk
GSPLIT = 4  # gather instructions per super-chunk
B0 = 68    # layer-0 per-head block: x(64) | ones | pad(3)


def _bcast_mid(ap_sl, reps):
    """[128, F] -> [128, reps, F] broadcasting the middle axis."""
    return bass.AP(ap_sl.tensor, ap_sl.offset,
                   [list(ap_sl.ap[0]), [0, reps], list(ap_sl.ap[-1])])


def _bcast_last(ap_sl, reps):
    """[128, M] -> [128, M, reps] broadcasting the last axis."""
    return bass.AP(ap_sl.tensor, ap_sl.offset,
                   [list(ap_sl.ap[0]), list(ap_sl.ap[-1]), [0, reps]])


def _build_program(NP, F_IN, HC, H, C, NT, K_t, FTS, GS, hi_q,
                   use_b0, use_b1, use_l0b, use_l1b):
    NCHUNK = int(sum(K_t))
    KMAX = max(K_t)
    E_pad = NCHUNK * 128
    SW = E_pad // 16
    TW = HC + 128          # L1 table row: h | asrc,adst (f32 bits) | pad
    KB = HC // 128
    NAG = len(GS)          # allgather groups (tile counts, uneven ok)
    RG = [0]
    for gsz in GS:
        RG.append(RG[-1] + gsz)
    g_of_t = []
    for gi, gsz in enumerate(GS):
        g_of_t += [gi] * gsz

    nc = bacc.Bacc(dynamic_dma_scratch_size=65536, num_swdge_queues=4)
    P = nc.declare_dram_parameter

    xgs = P("xgs", [128, NCHUNK * 80], BF16, isOutput=False)
    r1h = P("r1h", [HC, HC], BF16, isOutput=False)
    r1a = P("r1a", [HC, 8], BF16, isOutput=False)
    r2 = P("r2", [HC, FTS], BF16, isOutput=False)
    r3 = P("r3", [FTS, 1], BF16, isOutput=False)
    w0h = P("w0h", [64, HC], BF16, isOutput=False)
    b1t = P("b1t", [128, HC], F32, isOutput=False)
    l0bt = P("l0bt", [128, FTS], F32, isOutput=False)
    l1bt = P("l1bt", [128, 1], F32, isOutput=False)
    ident = P("ident", [128, 128], F32, isOutput=False)
    identb = P("identb", [128, 128], BF16, isOutput=False)
    srcw = P("srcw", [128, SW], I16, isOutput=False)
    ohb = P("ohb", [128, NCHUNK * 128], FP8, isOutput=False)
    oht = P("oht", [128, NCHUNK * 128], FP8, isOutput=False)
    ae1 = P("ae1", [128, NCHUNK, 4], BF16, isOutput=False)
    adt0 = P("adt0", [128, NT * 4], BF16, isOutput=False)
    outp = P("out", [NT * 128, 1], F32, isOutput=True)

    # chunk q -> owning dst tile
    t_of_q = []
    for t in range(NT):
        t_of_q += [t] * K_t[t]

    with tile.TileContext(nc) as tc:
        with (
            tc.tile_pool(name="const", bufs=1) as const,
            tc.tile_pool(name="stage", bufs=3) as stage,
            tc.tile_pool(name="work", bufs=3) as work,
            tc.tile_pool(name="gpp", bufs=6) as gpp,
            tc.tile_pool(name="tp", bufs=6) as tp,
            tc.tile_pool(name="psacc", bufs=2, space="PSUM") as psacc,
            tc.tile_pool(name="psfin", bufs=1, space="PSUM") as psfin,
            tc.tile_pool(name="pss", bufs=1, space="PSUM") as pss,
            tc.tile_pool(name="pstr", bufs=2, space="PSUM") as pstr,
            tc.tile_pool(name="pspd", bufs=2, space="PSUM") as pspd,
            tc.tile_pool(name="dram", bufs=1, space="DRAM") as dram,
        ):
            AGW = TW            # full-width AG rows (direct into H1)
            H1 = dram.tile([NP, TW], BF16, tag="H1")
            H1g = [dram.tile([GS[g] * 128, AGW], BF16, tag=f"H1g{g}",
                             name=f"H1g{g}")
                   for g in range(NAG)]

            _cn = [0]

            def cload(ap_in, shape, dt=F32, tag=None):
                _cn[0] += 1
                cname = tag or f"c{_cn[0]}"
                t = const.tile(shape, dt, tag=cname, name=f"{cname}_{_cn[0]}")
                nc.sync.dma_start(out=t[:], in_=ap_in)
                return t

            r1h_s = [cload(r1h[k * 128:(k + 1) * 128, :], [128, HC], BF16)
                     for k in range(KB)]
            r1a_s = [cload(r1a[k * 128:(k + 1) * 128, :], [128, 8], BF16)
                     for k in range(KB)]
            r2_s = [cload(r2[k * 128:(k + 1) * 128, :], [128, FTS], BF16)
                    for k in range(KB)]
            r3_s = cload(r3[:, :], [FTS, 1], BF16)
            w0h_s = cload(w0h[:, :], [64, HC], BF16)
            l0b_s = cload(l0bt[:, :], [128, FTS]) if use_l0b else None
            l1b_s = cload(l1bt[:, :], [128, 1]) if use_l1b else None
            b1_s = cload(b1t[:, :], [128, HC]) if use_b1 else None
            id_s = cload(ident[:, :], [128, 128])
            idb_s = cload(identb[:, :], [128, 128], BF16)
            srcw_s = cload(srcw[:, :], [128, SW], I16)
            ae1_s = cload(ae1[:, :, :], [128, NCHUNK, 4], BF16, tag="ae1")
            adt0_s = cload(adt0[:, :], [128, NT * 4], BF16, tag="adt0")
            # L1 adst rows, written by fin0 tile by tile
            adt1_s = const.tile([128, NT * 4], BF16, tag="adt1", name="adt1")
            # adst expanded per edge-slot, precomputed out of the hot loop
            pd0_all = const.tile([128, NCHUNK, 4], BF16, tag="pd0", name="pd0")
            pd1_all = const.tile([128, NCHUNK, 4], BF16, tag="pd1", name="pd1")

            # chunk ranges per tile
            qstart = [0]
            for t in range(NT):
                qstart.append(qstart[-1] + K_t[t])

            def pead_tile(t, adt_s, pd_all, ae_s=None):
                """adst-expand all chunks of tile t via oht matmuls."""
                kt = K_t[t]
                q0 = qstart[t]
                ot = stage.tile([128, KMAX * 128], FP8, tag="ohtp",
                                name="ohtp", bufs=2)
                nc.sync.dma_start(out=ot[:, 0:kt * 128],
                                  in_=oht[:, q0 * 128:(q0 + kt) * 128])
                pdp = pspd.tile([128, KMAX * 4], F32, tag="pd", name="pd")
                for k in range(kt):
                    nc.tensor.matmul(
                        pdp[:, k * 4:(k + 1) * 4],
                        ot[:, k * 128:(k + 1) * 128],
                        adt_s[:, t * 4:(t + 1) * 4],
                        start=True, stop=True)
                pdv = pdp[:, 0:kt * 4].rearrange("x (a b) -> x a b", b=4)
                if ae_s is None:
                    nc.vector.tensor_copy(pd_all[:, q0:q0 + kt, :], pdv)
                else:
                    # fold the per-edge ae term in here (saves an add in
                    # the hot per-super chain)
                    nc.vector.tensor_add(pd_all[:, q0:q0 + kt, :], pdv,
                                         ae_s[:, q0:q0 + kt, :])

            def agg_layer(lname, elem, pd_all, finalize, pre_tile=None):
                def ensure_super(s):
                    cnt = min(SCC * 128, E_pad - s * SCC * 128)
                    nch = cnt // 128
                    g = stage.tile([128, SCC, elem], BF16,
                                   tag=f"g{lname}", name=f"g{lname}",
                                   bufs=(4 if lname == "l0" else 5))
                    if lname == "l0":
                        # host pre-gathered edge stream, plain DMA
                        nc.sync.dma_start(
                            out=g[:, 0:nch, :],
                            in_=xgs[:, s * SCC * 80:(s * SCC + nch) * 80])
                    else:
                        c0 = s * SCC * 8
                        nq = min(GSPLIT, nch)
                        base = 0
                        for qi in range(nq):
                            take = (nch - base + (nq - qi) - 1) // (nq - qi)
                            # rows are src-sorted per tile: this split only
                            # reads H1[:hi], so its gather unlocks as soon
                            # as the AllGather groups covering those rows
                            # have landed (overlaps the AG tail)
                            hi = int(max(hi_q[s * SCC + base:
                                             s * SCC + base + take]))
                            nc.gpsimd.dma_gather(
                                g[:, base:base + take, :], H1[0:hi, :],
                                srcw_s[:, c0 + base * 8:
                                       c0 + (base + take) * 8],
                                take * 128, take * 128, elem,
                                single_packet=True,
                                queue_num=(s * nq + qi) % 4)
                            base += take
                    oh = stage.tile([128, SCC, 128], FP8, tag="oh",
                                    name="oh", bufs=6)
                    nc.sync.dma_start(
                        out=oh[:, 0:nch, :],
                        in_=ohb[:, s * SCC * 128:(s * SCC + nch) * 128])
                    if lname == "l0":
                        # asrc0 + ae0 pre-folded on host into the stream
                        t0v = g[:, 0:nch, 68:76].bitcast(F32)
                    else:
                        # ae1 folded into pd_all at pead time
                        t0v = g[:, 0:nch, HC:HC + 8].bitcast(F32)
                    t1 = work.tile([128, SCC, 4], F32, tag="t1", bufs=6)
                    nc.vector.tensor_add(
                        t1[:, 0:nch, :], t0v,
                        pd_all[:, s * SCC:s * SCC + nch, :])
                    t3 = work.tile([128, SCC, 4], F32, tag="t3", bufs=6)
                    nc.scalar.activation(t3[:, 0:nch, :], t1[:, 0:nch, :],
                                         AF.Prelu, alpha=0.2)
                    pb = work.tile([128, SCC, 4], BF16, tag="pb", bufs=6)
                    nc.scalar.activation(pb[:, 0:nch, :], t3[:, 0:nch, :],
                                         AF.Exp)
                    # p-weighted gather rows for the whole super, one DVE op
                    BW = B0 if lname == "l0" else C
                    gps = gpp.tile([128, SCC, 4, BW], BF16,
                                   tag=f"gps{lname}", name=f"gps{lname}",
                                   bufs=3)
                    # two halves so downstream matmuls start sooner
                    h1n = min(nch, SCC // 2)
                    for (a, b) in ((0, h1n), (h1n, nch)):
                        if b <= a:
                            continue
                        if lname == "l0":
                            sl = g[:, a:b, 0:BW]
                            i0 = bass.AP(sl.tensor, sl.offset,
                                         [list(sl.ap[0]), list(sl.ap[1]),
                                          [0, 4], list(sl.ap[2])])
                        else:
                            i0 = g[:, a:b, 0:HC].rearrange(
                                "x a (h c) -> x a h c", h=H)
                        psl = pb[:, a:b, :]
                        i1 = bass.AP(psl.tensor, psl.offset,
                                     [list(psl.ap[0]), list(psl.ap[1]),
                                      list(psl.ap[2]), [0, BW]])
                        nc.vector.tensor_mul(gps[:, a:b, :, :], i0, i1)
                    return gps, oh, pb

                q = 0
                gps = oh = pb = None
                BW = B0 if lname == "l0" else C
                for t in range(NT):
                    if pre_tile is not None:
                        pre_tile(t)
                    if lname == "l0":
                        ps_o = psacc.tile([128, 4 * B0], F32, tag="ps")
                        ps_s = None
                    else:
                        ps_o = psacc.tile([128, HC], F32, tag="ps")
                        ps_s = pss.tile([128, 8], F32, tag="pss")
                    for k in range(K_t[t]):
                        s, j = divmod(q, SCC)
                        if j == 0:
                            gps, oh, pb = ensure_super(s)
                        first, last = (k == 0), (k == K_t[t] - 1)
                        oh_j = oh[:, j, :]
                        rhs = gps[:, j, :, :].rearrange(
                            "x h c -> x (h c)")
                        nc.tensor.matmul(ps_o[:], oh_j, rhs,
                                         start=first, stop=last)
                        if lname != "l0":
                            nc.tensor.matmul(ps_s[:, 0:4], oh_j, pb[:, j, :],
                                             start=first, stop=last)
                        q += 1
                    finalize(t, ps_o, ps_s)

            # ---- layer-0 finalize: normalize in x-space, apply W0, relu,
            #      layer-1 linear, H1 group AllGather ----
            def fin0(t, ps_o, ps_s):
                po = ps_o[:].rearrange("x (h c) -> x h c", h=H)
                sp = work.tile([128, 4], F32, tag="sp")
                nc.vector.tensor_scalar_add(sp[:], po[:, :, 64], 1e-16)
                rc = work.tile([128, 4], F32, tag="rc")
                nc.vector.reciprocal(rc[:], sp[:])
                ax = work.tile([128, 4, 64], BF16, tag="ax")
                nc.vector.tensor_mul(ax[:], po[:, :, 0:64],
                                     _bcast_last(rc[:], 64))
                a0k = []
                for h in range(H):
                    ptb = pstr.tile([64, 128], BF16, tag="pt")
                    nc.tensor.transpose(ptb[:], ax[:, h, :], idb_s[:])
                    ak = tp.tile([64, 128], BF16, tag="axT", name=f"axT{h}")
                    nc.vector.tensor_copy(ak[:], ptb[:])
                    a0k.append(ak)
                out0 = psfin.tile([128, HC], F32, tag="pf")
                for h in range(H):
                    nc.tensor.matmul(out0[:, h * 128:(h + 1) * 128],
                                     a0k[h][:], w0h_s[:, h * 128:(h + 1) * 128],
                                     start=True, stop=True)
                ar = work.tile([128, HC], BF16, tag="ar", bufs=2)
                nc.vector.tensor_scalar_max(ar[:], out0[:], 0.0)
                a1 = []
                for kk in range(KB):
                    pt = pstr.tile([128, 128], BF16, tag="pt")
                    nc.tensor.transpose(pt[:], ar[:, kk * 128:(kk + 1) * 128],
                                        idb_s[:])
                    ak = tp.tile([128, 128], BF16, tag="a1T", name=f"a0k{kk}")
                    nc.vector.tensor_copy(ak[:], pt[:])
                    a1.append(ak)
                ph1 = psfin.tile([128, HC], F32, tag="pf")
                pa1 = pss.tile([128, 8], F32, tag="pss")
                for kk in range(KB):
                    first, last = (kk == 0), (kk == KB - 1)
                    nc.tensor.matmul(ph1[:], a1[kk][:], r1h_s[kk][:],
                                     start=first, stop=last)
                    nc.tensor.matmul(pa1[:], a1[kk][:], r1a_s[kk][:],
                                     start=first, stop=last)
                # stash adst1 (bf16), then expand it for tile t's edge slots
                nc.vector.tensor_copy(adt1_s[:, t * 4:(t + 1) * 4],
                                      pa1[:, 4:8])
                pead_tile(t, adt1_s, pd1_all, ae_s=ae1_s)
                st = stage.tile([128, AGW], BF16, tag="hrow")
                if t % 2 == 0:
                    nc.vector.tensor_copy(st[:, 0:HC], ph1[:])
                else:
                    nc.scalar.activation(st[:, 0:HC], ph1[:], AF.Copy)
                nc.scalar.activation(st[:, HC:HC + 16].bitcast(F32),
                                     pa1[:], AF.Copy)
                gidx = g_of_t[t]
                loc = t - RG[gidx]
                nc.sync.dma_start(out=H1g[gidx][loc * 128:(loc + 1) * 128, :],
                                  in_=st[:])
                if loc == GS[gidx] - 1:
                    r0 = RG[gidx] * NCORES * 128
                    r1 = RG[gidx + 1] * NCORES * 128
                    nc.gpsimd.collective_compute(
                        "AllGather", OP.bypass,
                        replica_groups=[list(range(NCORES))],
                        ins=[H1g[gidx].opt()],
                        outs=[H1[r0:r1, :].opt()],
                    )

            # ---- layer-1 finalize: normalize + relu + MLP head ----
            def fin1(t, ps_o, ps_s):
                sp = work.tile([128, 4], F32, tag="sp")
                nc.vector.tensor_scalar_add(sp[:], ps_s[:, 0:4], 1e-16)
                rc = work.tile([128, 4], F32, tag="rc")
                nc.vector.reciprocal(rc[:], sp[:])
                ao = work.tile([128, HC], F32, tag="ao", bufs=2)
                nc.vector.tensor_mul(
                    ao[:].rearrange("x (h c) -> x h c", h=H),
                    ps_o[:].rearrange("x (h c) -> x h c", h=H),
                    _bcast_last(rc[:], C))
                if use_b1:
                    ab = work.tile([128, HC], F32, tag="ao", bufs=2)
                    nc.vector.tensor_add(ab[:], ao[:], b1_s[:])
                    ao2 = ab
                else:
                    ao2 = ao
                ar = work.tile([128, HC], BF16, tag="ar1", bufs=2)
                nc.vector.tensor_scalar_max(ar[:], ao2[:], 0.0)
                h2p = psfin.tile([128, FTS], F32, tag="pf")
                for kk in range(KB):
                    pt = pstr.tile([128, 128], BF16, tag="pt")
                    nc.tensor.transpose(pt[:], ar[:, kk * 128:(kk + 1) * 128],
                                        idb_s[:])
                    a1k = tp.tile([128, 128], BF16, tag="a1T32")
                    nc.scalar.activation(a1k[:], pt[:], AF.Copy)
                    nc.tensor.matmul(h2p[:], a1k[:], r2_s[kk][:],
                                     start=(kk == 0), stop=(kk == KB - 1))
                if use_l0b:
                    h2b = work.tile([128, FTS], F32, tag="h2b")
                    nc.vector.tensor_add(h2b[:], h2p[:], l0b_s[:])
                else:
                    h2b = h2p
                h2r = work.tile([128, FTS], BF16, tag="h2r")
                nc.vector.tensor_scalar_max(h2r[:], h2b[:], 0.0)
                pt2 = pstr.tile([128, 128], BF16, tag="pt")
                nc.tensor.transpose(pt2[:], h2r[:], idb_s[:])
                h2T = tp.tile([128, 128], BF16, tag="a1T32")
                nc.scalar.activation(h2T[:], pt2[:], AF.Copy)
                po = pss.tile([128, 8], F32, tag="pss")
                nc.tensor.matmul(po[:, 0:1], h2T[:], r3_s[:],
                                 start=True, stop=True)
                ob = work.tile([128, 1], F32, tag="ob")
                if use_l1b:
                    nc.vector.tensor_add(ob[:], po[:, 0:1], l1b_s[:])
                else:
                    nc.vector.tensor_copy(ob[:], po[:, 0:1])
                nc.sync.dma_start(out=outp[t * 128:(t + 1) * 128, :],
                                  in_=ob[:])

            # pead for layer 0 is pipelined into the aggregation loop:
            # before tile t's chunks run, peads are emitted for every
            # tile any super ensured during t can touch
            pead_next = [0]

            def pre0(t):
                s_last = (qstart[t + 1] - 1) // SCC
                last_chunk = min((s_last + 1) * SCC - 1, NCHUNK - 1)
                tl = t_of_q[last_chunk]
                while pead_next[0] <= tl:
                    pead_tile(pead_next[0], adt0_s, pd0_all)
                    pead_next[0] += 1

            nc._state.push_named_scope("phaseB")
            agg_layer("l0", 80, pd0_all, fin0, pre_tile=pre0)
            nc._state.pop_named_scope("phaseB")
            nc._state.push_named_scope("phaseD")
            agg_layer("l1", TW, pd1_all, fin1)
            nc._state.pop_named_scope("phaseD")

    nc.finalize()
    return nc


def _wrap_idx(v, E_pad):
    blk = np.zeros((16, E_pad // 16), np.int16)
    ar = np.arange(E_pad)
    blk[ar % 16, ar // 16] = v.astype(np.int16)
    return np.tile(blk, (8, 1))


def kernel(x, edge_index, edge_weights,
           W0, as0, ad0, We0, ae0, b0,
           W1, as1, ad1, We1, ae1, b1,
           L0W, L0b, L1W, L1b):
    x = np.asarray(x, np.float32)
    N, F_IN = x.shape
    HC = W0.shape[0]
    H, C = np.asarray(as0).shape
    FTS = np.asarray(L0W).shape[0]

    NT = -(-N // (128 * NCORES))
    SHARD = NT * 128
    NP = SHARD * NCORES
    # allgather groups (tile counts): sized so each group's transfer keeps
    # pace with layer-0 tile production; 1-tile tail minimizes the
    # phase-boundary stall
    if NT == 20:
        GS = [4, 4, 4, 3, 2, 2, 1]
    else:
        GS = [NT]
    RG = np.zeros(len(GS) + 1, np.int64)
    RG[1:] = np.cumsum(GS)
    g_of_t = np.repeat(np.arange(len(GS)), GS)

    # ---- edges ----
    ew_in = np.asarray(edge_weights, np.float32)
    src = np.concatenate([np.asarray(edge_index[0]), np.arange(N)])
    dst = np.concatenate([np.asarray(edge_index[1]), np.arange(N)])
    ew = np.concatenate([ew_in, np.full(N, ew_in.mean(), np.float32)])

    # ---- degree-balanced node -> (core, tile, slot) assignment ----
    # LPT-pack nodes into NTG bins of 128 by in-degree, then deal bins to
    # (tile, core) rank-major so per-tile maxima (=> K_t padding) equalize
    NTG = NP // 128
    nodes = np.arange(NP)
    deg = np.bincount(dst, minlength=NP)
    import heapq
    heap = [(0, b) for b in range(NTG)]
    heapq.heapify(heap)
    bincnt = np.zeros(NTG, np.int64)
    binsum = np.zeros(NTG, np.int64)
    bin_of_n = np.empty(NP, np.int64)
    for n in np.argsort(-deg, kind="stable"):
        while True:
            s, b = heapq.heappop(heap)
            if bincnt[b] < 128:
                break
        bin_of_n[n] = b
        binsum[b] += deg[n]
        bincnt[b] += 1
        if bincnt[b] < 128:
            heapq.heappush(heap, (int(binsum[b]), b))
    brank = np.argsort(-binsum, kind="stable")
    tile_of_bin = np.empty(NTG, np.int64)
    core_of_bin = np.empty(NTG, np.int64)
    tile_of_bin[brank] = np.arange(NTG) // NCORES
    core_of_bin[brank] = np.arange(NTG) % NCORES
    n_tile = tile_of_bin[bin_of_n]
    n_core = core_of_bin[bin_of_n]
    gt_of_n = n_core * NT + n_tile               # node -> global tile
    ord2 = np.argsort(gt_of_n, kind="stable")
    n_slot = np.empty(NP, np.int64)
    n_slot[ord2] = np.arange(NP) % 128
    node_of = np.empty(NP, np.int64)             # (gtile*128+slot) -> node
    node_of[gt_of_n * 128 + n_slot] = nodes

    # table-row permutation (group-major) so group AllGathers land contiguous
    gg = g_of_t[n_tile]
    t_of_n = (RG[gg] * NCORES * 128 + n_core * np.asarray(GS)[gg] * 128
              + (n_tile - RG[gg]) * 128 + n_slot)    # node -> table row

    order = np.argsort(gt_of_n[dst], kind="stable")
    src_s, dst_s, ew_s = src[order], dst[order], ew[order]

    tile_of = gt_of_n[dst_s]
    tcounts = np.bincount(tile_of, minlength=NTG)
    tstart = np.concatenate([[0], np.cumsum(tcounts)])

    K_t = [max(1, int(max(-(-tcounts[i * NT + t] // 128)
                          for i in range(NCORES))))
           for t in range(NT)]
    NCHUNK = int(sum(K_t))
    E_pad = NCHUNK * 128

    # ---- weight folding (host, O(weights) + O(N*F_IN*H)) ----
    as0 = np.asarray(as0, np.float32)
    ad0 = np.asarray(ad0, np.float32)
    ae0w = np.asarray(ae0, np.float32)
    as1 = np.asarray(as1, np.float32)
    ad1 = np.asarray(ad1, np.float32)
    ae1w = np.asarray(ae1, np.float32)
    W0 = np.asarray(W0, np.float32)
    W1 = np.asarray(W1, np.float32)
    We0 = np.asarray(We0, np.float32)
    We1 = np.asarray(We1, np.float32)

    k0 = (We0.reshape(H, C) * ae0w).sum(1).astype(np.float32)
    k1 = (We1.reshape(H, C) * ae1w).sum(1).astype(np.float32)

    def fold(W, a):
        blk = np.zeros((HC, H), np.float32)
        for h in range(H):
            blk[h * C:(h + 1) * C, h] = a[h]
        return (W.T @ blk).astype(np.float32)

    bf = ml_dtypes.bfloat16
    r1h = W1.T.astype(bf)
    r1a = np.concatenate([fold(W1, as1), fold(W1, ad1)], 1).astype(bf)
    r2 = np.asarray(L0W, np.float32).T.astype(bf)
    r3 = np.asarray(L1W, np.float32).T.astype(bf)
    w0h = W0.T.astype(bf)           # [64, 512]; cols h*128.. = W0_h^T

    # per-node layer-0 attention terms (tiny host matmuls)
    asrc0 = (x @ fold(W0, as0)).astype(np.float32)   # [N, 4]
    adst0 = (x @ fold(W0, ad0)).astype(np.float32)   # [N, 4]

    xbf = x.astype(bf)                               # node-order x, bf16
    adsta = np.zeros((NP, 4), np.float32)
    adsta[:N] = adst0

    b1t = np.tile(np.asarray(b1, np.float32)[None, :], (128, 1))
    l0bt = np.tile(np.asarray(L0b, np.float32)[None, :], (128, 1))
    l1bt = np.tile(np.asarray(L1b, np.float32).reshape(1, 1), (128, 1))
    ident = np.eye(128, dtype=np.float32)
    identb = np.eye(128, dtype=np.float32).astype(bf)

    in_maps = []
    srcp_all = []
    for i in range(NCORES):
        srcp = np.zeros(E_pad, np.int64)
        srcn = np.zeros(E_pad, np.int64)         # node-id src (host gather)
        dlocp = np.full(E_pad, -1, np.int64)
        ewp = np.zeros(E_pad, np.float32)
        offq = 0
        for t in range(NT):
            gt = i * NT + t
            cnt = int(tcounts[gt])
            sl = slice(tstart[gt], tstart[gt] + cnt)
            # order tile's edges by src table row: the L1 gather then
            # walks H1 monotonically (better HBM behavior, dups adjacent)
            so = np.argsort(t_of_n[src_s[sl]], kind="stable")
            srcp[offq:offq + cnt] = t_of_n[src_s[sl]][so]
            srcn[offq:offq + cnt] = src_s[sl][so]
            dlocp[offq:offq + cnt] = n_slot[dst_s[sl]][so]
            ewp[offq:offq + cnt] = ew_s[sl][so]
            offq += K_t[t] * 128
        srcp_all.append(srcp.copy())
        ae1p = (ewp[:, None] * k1[None, :]).reshape(
            NCHUNK, 128, 4).transpose(1, 0, 2)
        # layer-0 edge stream: [x(64) | 1 | 0*3 | asrc0+ae0 f32 bits | pad]
        xg_u16 = np.zeros((E_pad, 80), np.uint16)
        xg_u16[:, 0:64] = xbf[srcn].view(np.uint16)
        xg_u16[:, 64] = np.array(1.0, bf).view(np.uint16)
        l0a = (asrc0[srcn] + ewp[:, None] * k0[None, :]).astype(np.float32)
        xg_u16[:, 68:76] = np.ascontiguousarray(l0a).view(np.uint16)
        xgs_np = np.ascontiguousarray(
            xg_u16.reshape(NCHUNK, 128, 80).transpose(1, 0, 2)
            .reshape(128, NCHUNK * 80)).view(bf)
        dl2 = dlocp.reshape(NCHUNK, 128)
        valid = dl2 >= 0
        qs, es = np.nonzero(valid)
        f8 = ml_dtypes.float8_e4m3fn
        # ohb[e, q, d]: partition = edge-slot e
        ohb_np = np.zeros((128, NCHUNK, 128), f8)
        ohb_np[es, qs, dl2[qs, es]] = 1.0
        ohb_np = np.ascontiguousarray(ohb_np.reshape(128, NCHUNK * 128))
        # oht[d, q, e]: partition = dst-slot d
        oht_np = np.zeros((128, NCHUNK, 128), f8)
        oht_np[dl2[qs, es], qs, es] = 1.0
        oht_np = np.ascontiguousarray(oht_np.reshape(128, NCHUNK * 128))
        # adst0 rows for this core's dst tiles
        adt0_np = np.zeros((128, NT * 4), np.float32)
        for t in range(NT):
            rows = node_of[(i * NT + t) * 128 + np.arange(128)]
            adt0_np[:, t * 4:(t + 1) * 4] = adsta[rows]
        in_maps.append({
            "xgs": xgs_np, "r1h": r1h, "r1a": r1a, "r2": r2, "r3": r3,
            "w0h": w0h, "b1t": b1t, "l0bt": l0bt, "l1bt": l1bt,
            "ident": ident, "identb": identb,
            "srcw": _wrap_idx(srcp, E_pad),
            "ohb": ohb_np, "oht": oht_np,
            "ae1": np.ascontiguousarray(ae1p).astype(bf),
            "adt0": adt0_np.astype(bf),
        })

    hi_q = (np.stack(srcp_all).reshape(NCORES, NCHUNK, 128)
            .max(axis=(0, 2)) + 1)
    nc = _build_program(NP, F_IN, HC, H, C, NT, K_t, FTS, GS, hi_q,
                        bool(np.any(b0)), bool(np.any(b1)),
                        bool(np.any(np.asarray(L0b))),
                        bool(np.any(np.asarray(L1b))))
    res = run_bass_kernel_spmd(nc, in_maps, list(range(NCORES)))
    cat = np.concatenate([res.results[i]["out"][:, 0] for i in range(NCORES)])
    out_full = np.empty(NP, np.float32)
    out_full[node_of] = cat
    return out_full[:N].astype(np.float32)



# revision 34
# speedup vs baseline: 1.2283x; 1.1608x over previous
"""2-layer GAT + MLP head on 8 TRN2 NeuronCores.

Strategy (dst-sharded, layer-0 aggregation folded into x-space):
- Nodes padded to NP=20480; each core owns a contiguous 2560-dst shard.
  Edges (incl. self-loops, PyG mean-fill edge attr) sorted by dst,
  grouped into 128-dst tiles, padded per tile-slot to a chunk count K_t
  shared by all cores (SPMD: one program).
- Layer 0 aggregates RAW x per head (W0 applied after the softmax
  average, per dst tile): the gather table is x padded to 128 bf16 cols
  [x(64) | 1 | 0*3 | asrc0 f32-bits(8) | 0...]; the ones column makes
  the softmax denominator fall out of the same one-hot matmul.
  asrc0/adst0 are host-folded (O(N*F_IN*H) prep).
- Layer 1 gathers 1280B rows of the H1 table [h1 bf16(512) | asrc1,
  adst1 f32-bits(16) | pad]; H1 is assembled by NAG group-wise
  AllGathers (Shared scratchpad) that overlap layer-0 finalize.
- Per 128-edge chunk: p = exp(leakyrelu(asrc+adst+ae)) batched per
  super-chunk; adst expanded per edge via a one-hot-transpose matmul
  (adst rows live in SBUF: host const for L0, fin0-written for L1);
  out[dst] accumulated via one-hot matmuls in PSUM.
- ohb/oht one-hot blocks stream from HBM as one packed bf16 tensor.
"""

import numpy as np
import ml_dtypes

import concourse.bacc as bacc
import concourse.bass as bass
import concourse.mybir as mybir
import concourse.tile as tile
from concourse.bass_utils import run_bass_kernel_spmd

F32 = mybir.dt.float32
BF16 = mybir.dt.bfloat16
FP8 = mybir.dt.float8e4
I16 = mybir.dt.int16
AF = mybir.ActivationFunctionType
OP = mybir.AluOpType

NCORES = 8
SCC = 8    # chunks (of 128 edges) per gather super-chunk
GSPLIT = 4  # gather instructions per super-chunk
B0 = 68    # layer-0 per-head block: x(64) | ones | pad(3)


def _bcast_mid(ap_sl, reps):
    """[128, F] -> [128, reps, F] broadcasting the middle axis."""
    return bass.AP(ap_sl.tensor, ap_sl.offset,
                   [list(ap_sl.ap[0]), [0, reps], list(ap_sl.ap[-1])])


def _bcast_last(ap_sl, reps):
    """[128, M] -> [128, M, reps] broadcasting the last axis."""
    return bass.AP(ap_sl.tensor, ap_sl.offset,
                   [list(ap_sl.ap[0]), list(ap_sl.ap[-1]), [0, reps]])


def _build_program(NP, F_IN, HC, H, C, NT, K_t, FTS, GS, hi_q,
                   seq, tile_slots,
                   use_b0, use_b1, use_l0b, use_l1b):
    NCHUNK = int(sum(K_t))
    KMAX = max(K_t)
    E_pad = NCHUNK * 128
    SW = E_pad // 16
    TW = HC + 128          # L1 table row: h | asrc,adst (f32 bits) | pad
    KB = HC // 128
    NAG = len(GS)          # allgather groups (tile counts, uneven ok)
    RG = [0]
    for gsz in GS:
        RG.append(RG[-1] + gsz)
    g_of_t = []
    for gi, gsz in enumerate(GS):
        g_of_t += [gi] * gsz

    nc = bacc.Bacc(dynamic_dma_scratch_size=65536, num_swdge_queues=4)
    P = nc.declare_dram_parameter

    xgs = P("xgs", [128, NCHUNK * 80], BF16, isOutput=False)
    r1h = P("r1h", [HC, HC], BF16, isOutput=False)
    r1a = P("r1a", [HC, 8], BF16, isOutput=False)
    r2 = P("r2", [HC, FTS], BF16, isOutput=False)
    r3 = P("r3", [FTS, 1], BF16, isOutput=False)
    w0h = P("w0h", [64, HC], BF16, isOutput=False)
    b1t = P("b1t", [128, HC], F32, isOutput=False)
    l0bt = P("l0bt", [128, FTS], F32, isOutput=False)
    l1bt = P("l1bt", [128, 1], F32, isOutput=False)
    ident = P("ident", [128, 128], F32, isOutput=False)
    identb = P("identb", [128, 128], BF16, isOutput=False)
    srcw = P("srcw", [128, SW], I16, isOutput=False)
    ohb = P("ohb", [128, NCHUNK * 128], FP8, isOutput=False)
    oht = P("oht", [128, NCHUNK * 128], FP8, isOutput=False)
    ae1 = P("ae1", [128, NCHUNK, 4], BF16, isOutput=False)
    adt0 = P("adt0", [128, NT * 4], BF16, isOutput=False)
    outp = P("out", [NT * 128, 1], F32, isOutput=True)

    # chunk q -> owning dst tile (pair-interleaved schedule)
    t_of_q = [t for (t, k) in seq]
    SL0 = [tile_slots[t][0] for t in range(NT)]
    SSTR = [(tile_slots[t][1] - tile_slots[t][0]) if K_t[t] > 1 else 1
            for t in range(NT)]
    for t in range(NT):
        d = np.diff(tile_slots[t])
        assert len(d) == 0 or (d == d[0]).all(), "irregular slot stride"


    with tile.TileContext(nc) as tc:
        with (
            tc.tile_pool(name="const", bufs=1) as const,
            tc.tile_pool(name="stage", bufs=3) as stage,
            tc.tile_pool(name="work", bufs=3) as work,
            tc.tile_pool(name="gpp", bufs=6) as gpp,
            tc.tile_pool(name="tp", bufs=6) as tp,
            tc.tile_pool(name="psacc", bufs=2, space="PSUM") as psacc,
            tc.tile_pool(name="psfin", bufs=1, space="PSUM") as psfin,
            tc.tile_pool(name="pss", bufs=1, space="PSUM") as pss,
            tc.tile_pool(name="pstr", bufs=2, space="PSUM") as pstr,
            tc.tile_pool(name="pspd", bufs=2, space="PSUM") as pspd,
            tc.tile_pool(name="dram", bufs=1, space="DRAM") as dram,
        ):
            AGW = TW            # full-width AG rows (direct into H1)
            H1 = dram.tile([NP, TW], BF16, tag="H1")
            H1g = [dram.tile([GS[g] * 128, AGW], BF16, tag=f"H1g{g}",
                             name=f"H1g{g}")
                   for g in range(NAG)]

            _cn = [0]

            def cload(ap_in, shape, dt=F32, tag=None):
                _cn[0] += 1
                cname = tag or f"c{_cn[0]}"
                t = const.tile(shape, dt, tag=cname, name=f"{cname}_{_cn[0]}")
                nc.sync.dma_start(out=t[:], in_=ap_in)
                return t

            r1h_s = [cload(r1h[k * 128:(k + 1) * 128, :], [128, HC], BF16)
                     for k in range(KB)]
            r1a_s = [cload(r1a[k * 128:(k + 1) * 128, :], [128, 8], BF16)
                     for k in range(KB)]
            r2_s = [cload(r2[k * 128:(k + 1) * 128, :], [128, FTS], BF16)
                    for k in range(KB)]
            r3_s = cload(r3[:, :], [FTS, 1], BF16)
            w0h_s = cload(w0h[:, :], [64, HC], BF16)
            l0b_s = cload(l0bt[:, :], [128, FTS]) if use_l0b else None
            l1b_s = cload(l1bt[:, :], [128, 1]) if use_l1b else None
            b1_s = cload(b1t[:, :], [128, HC]) if use_b1 else None
            id_s = cload(ident[:, :], [128, 128])
            idb_s = cload(identb[:, :], [128, 128], BF16)
            srcw_s = cload(srcw[:, :], [128, SW], I16)
            ae1_s = cload(ae1[:, :, :], [128, NCHUNK, 4], BF16, tag="ae1")
            adt0_s = cload(adt0[:, :], [128, NT * 4], BF16, tag="adt0")
            # L1 adst rows, written by fin0 tile by tile
            adt1_s = const.tile([128, NT * 4], BF16, tag="adt1", name="adt1")
            # adst expanded per edge-slot, precomputed out of the hot loop
            pd0_all = const.tile([128, NCHUNK, 4], BF16, tag="pd0", name="pd0")
            pd1_all = const.tile([128, NCHUNK, 4], BF16, tag="pd1", name="pd1")

            # chunk ranges per tile
            qstart = [0]
            for t in range(NT):
                qstart.append(qstart[-1] + K_t[t])

            def pead_tile(t, adt_s, pd_all, ae_s=None):
                """adst-expand all chunks of tile t via oht matmuls."""
                kt = K_t[t]
                q0 = qstart[t]
                ot = stage.tile([128, KMAX * 128], FP8, tag="ohtp",
                                name="ohtp", bufs=2)
                nc.sync.dma_start(out=ot[:, 0:kt * 128],
                                  in_=oht[:, q0 * 128:(q0 + kt) * 128])
                pdp = pspd.tile([128, KMAX * 4], F32, tag="pd", name="pd")
                for k in range(kt):
                    nc.tensor.matmul(
                        pdp[:, k * 4:(k + 1) * 4],
                        ot[:, k * 128:(k + 1) * 128],
                        adt_s[:, t * 4:(t + 1) * 4],
                        start=True, stop=True)
                pdv = pdp[:, 0:kt * 4].rearrange("x (a b) -> x a b", b=4)

                def strided(tile_, kt_):
                    sl = tile_[:, SL0[t]:SL0[t] + kt_, :]
                    return bass.AP(sl.tensor, sl.offset,
                                   [list(sl.ap[0]),
                                    [sl.ap[1][0] * SSTR[t], kt_],
                                    list(sl.ap[2])])

                if ae_s is None:
                    nc.vector.tensor_copy(strided(pd_all, kt), pdv)
                else:
                    # fold the per-edge ae term in here (saves an add in
                    # the hot per-super chain)
                    nc.vector.tensor_add(strided(pd_all, kt), pdv,
                                         strided(ae_s, kt))

            def agg_layer(lname, elem, pd_all, finalize, pre_tile=None):
                def ensure_super(s):
                    cnt = min(SCC * 128, E_pad - s * SCC * 128)
                    nch = cnt // 128
                    g = stage.tile([128, SCC, elem], BF16,
                                   tag=f"g{lname}", name=f"g{lname}",
                                   bufs=(4 if lname == "l0" else 5))
                    if lname == "l0":
                        # host pre-gathered edge stream, plain DMA
                        nc.sync.dma_start(
                            out=g[:, 0:nch, :],
                            in_=xgs[:, s * SCC * 80:(s * SCC + nch) * 80])
                    else:
                        c0 = s * SCC * 8
                        nq = min(GSPLIT, nch)
                        base = 0
                        for qi in range(nq):
                            take = (nch - base + (nq - qi) - 1) // (nq - qi)
                            # rows are src-sorted per tile: this split only
                            # reads H1[:hi], so its gather unlocks as soon
                            # as the AllGather groups covering those rows
                            # have landed (overlaps the AG tail)
                            hi = int(max(hi_q[s * SCC + base:
                                             s * SCC + base + take]))
                            nc.gpsimd.dma_gather(
                                g[:, base:base + take, :], H1[0:hi, :],
                                srcw_s[:, c0 + base * 8:
                                       c0 + (base + take) * 8],
                                take * 128, take * 128, elem,
                                single_packet=True,
                                queue_num=(s * nq + qi) % 4)
                            base += take
                    oh = stage.tile([128, SCC, 128], FP8, tag="oh",
                                    name="oh", bufs=6)
                    nc.sync.dma_start(
                        out=oh[:, 0:nch, :],
                        in_=ohb[:, s * SCC * 128:(s * SCC + nch) * 128])
                    if lname == "l0":
                        # asrc0 + ae0 pre-folded on host into the stream
                        t0v = g[:, 0:nch, 68:76].bitcast(F32)
                    else:
                        # ae1 folded into pd_all at pead time
                        t0v = g[:, 0:nch, HC:HC + 8].bitcast(F32)
                    t1 = work.tile([128, SCC, 4], F32, tag="t1", bufs=6)
                    nc.vector.tensor_add(
                        t1[:, 0:nch, :], t0v,
                        pd_all[:, s * SCC:s * SCC + nch, :])
                    t3 = work.tile([128, SCC, 4], F32, tag="t3", bufs=6)
                    nc.scalar.activation(t3[:, 0:nch, :], t1[:, 0:nch, :],
                                         AF.Prelu, alpha=0.2)
                    pb = work.tile([128, SCC, 4], BF16, tag="pb", bufs=6)
                    nc.scalar.activation(pb[:, 0:nch, :], t3[:, 0:nch, :],
                                         AF.Exp)
                    # p-weighted gather rows for the whole super, one DVE op
                    BW = B0 if lname == "l0" else C
                    gps = gpp.tile([128, SCC, 4, BW], BF16,
                                   tag=f"gps{lname}", name=f"gps{lname}",
                                   bufs=3)
                    # two halves so downstream matmuls start sooner
                    h1n = min(nch, SCC // 2)
                    for (a, b) in ((0, h1n), (h1n, nch)):
                        if b <= a:
                            continue
                        if lname == "l0":
                            sl = g[:, a:b, 0:BW]
                            i0 = bass.AP(sl.tensor, sl.offset,
                                         [list(sl.ap[0]), list(sl.ap[1]),
                                          [0, 4], list(sl.ap[2])])
                        else:
                            i0 = g[:, a:b, 0:HC].rearrange(
                                "x a (h c) -> x a h c", h=H)
                        psl = pb[:, a:b, :]
                        i1 = bass.AP(psl.tensor, psl.offset,
                                     [list(psl.ap[0]), list(psl.ap[1]),
                                      list(psl.ap[2]), [0, BW]])
                        nc.vector.tensor_mul(gps[:, a:b, :, :], i0, i1)
                    return gps, oh, pb

                gps = oh = pb = None
                BW = B0 if lname == "l0" else C
                ps_open = {}
                pssT = [None]
                for q, (t, k) in enumerate(seq):
                    s, j = divmod(q, SCC)
                    if j == 0:
                        if pre_tile is not None:
                            pre_tile(max(t_of_q[s * SCC:
                                               min((s + 1) * SCC, NCHUNK)]))
                        gps, oh, pb = ensure_super(s)
                    if k == 0:
                        if lname == "l0":
                            ps_open[t] = (psacc.tile([128, 4 * B0], F32,
                                                     tag="ps",
                                                     name="ps0"), None)
                        else:
                            if t % 2 == 0:
                                pssT[0] = pss.tile([128, 16], F32, tag="pss",
                                                   name="pssT")
                            else:
                                # zero our half explicitly: the even tile's
                                # start=True may reset the whole bank, and
                                # our own accumulation must use start=False
                                # to never clobber the even tile's partials
                                nc.vector.memset(pssT[0][:, 8:16], 0.0)
                            half = pssT[0][:, (t % 2) * 8:(t % 2) * 8 + 8]
                            ps_open[t] = (psacc.tile([128, HC], F32,
                                                     tag="ps",
                                                     name="ps1"), half)
                    ps_o, ps_s = ps_open[t]
                    first, last = (k == 0), (k == K_t[t] - 1)
                    oh_j = oh[:, j, :]
                    rhs = gps[:, j, :, :].rearrange(
                        "x h c -> x (h c)")
                    nc.tensor.matmul(ps_o[:], oh_j, rhs,
                                     start=first, stop=last)
                    if lname != "l0":
                        nc.tensor.matmul(ps_s[:, 0:4], oh_j, pb[:, j, :],
                                         start=(first and t % 2 == 0),
                                         stop=last, skip_group_check=True)
                    if last:
                        finalize(t, ps_o, ps_s)
                        del ps_open[t]

            # ---- layer-0 finalize: normalize in x-space, apply W0, relu,
            #      layer-1 linear, H1 group AllGather ----
            def fin0(t, ps_o, ps_s):
                po = ps_o[:].rearrange("x (h c) -> x h c", h=H)
                sp = work.tile([128, 4], F32, tag="sp")
                nc.vector.tensor_scalar_add(sp[:], po[:, :, 64], 1e-16)
                rc = work.tile([128, 4], F32, tag="rc")
                nc.vector.reciprocal(rc[:], sp[:])
                ax = work.tile([128, 4, 64], BF16, tag="ax")
                nc.vector.tensor_mul(ax[:], po[:, :, 0:64],
                                     _bcast_last(rc[:], 64))
                a0k = []
                for h in range(H):
                    ptb = pstr.tile([64, 128], BF16, tag="pt")
                    nc.tensor.transpose(ptb[:], ax[:, h, :], idb_s[:])
                    ak = tp.tile([64, 128], BF16, tag="axT", name=f"axT{h}")
                    nc.vector.tensor_copy(ak[:], ptb[:])
                    a0k.append(ak)
                out0 = psfin.tile([128, HC], F32, tag="pf")
                for h in range(H):
                    nc.tensor.matmul(out0[:, h * 128:(h + 1) * 128],
                                     a0k[h][:], w0h_s[:, h * 128:(h + 1) * 128],
                                     start=True, stop=True)
                ar = work.tile([128, HC], BF16, tag="ar", bufs=2)
                nc.vector.tensor_scalar_max(ar[:], out0[:], 0.0)
                a1 = []
                for kk in range(KB):
                    pt = pstr.tile([128, 128], BF16, tag="pt")
                    nc.tensor.transpose(pt[:], ar[:, kk * 128:(kk + 1) * 128],
                                        idb_s[:])
                    ak = tp.tile([128, 128], BF16, tag="a1T", name=f"a0k{kk}")
                    nc.vector.tensor_copy(ak[:], pt[:])
                    a1.append(ak)
                ph1 = psfin.tile([128, HC], F32, tag="pf")
                pa1 = pss.tile([128, 8], F32, tag="pss")
                for kk in range(KB):
                    first, last = (kk == 0), (kk == KB - 1)
                    nc.tensor.matmul(ph1[:], a1[kk][:], r1h_s[kk][:],
                                     start=first, stop=last)
                    nc.tensor.matmul(pa1[:], a1[kk][:], r1a_s[kk][:],
                                     start=first, stop=last)
                # stash adst1 (bf16), then expand it for tile t's edge slots
                nc.vector.tensor_copy(adt1_s[:, t * 4:(t + 1) * 4],
                                      pa1[:, 4:8])
                pead_tile(t, adt1_s, pd1_all, ae_s=ae1_s)
                st = stage.tile([128, AGW], BF16, tag="hrow")
                if t % 2 == 0:
                    nc.vector.tensor_copy(st[:, 0:HC], ph1[:])
                else:
                    nc.scalar.activation(st[:, 0:HC], ph1[:], AF.Copy)
                nc.scalar.activation(st[:, HC:HC + 16].bitcast(F32),
                                     pa1[:], AF.Copy)
                gidx = g_of_t[t]
                loc = t - RG[gidx]
                nc.sync.dma_start(out=H1g[gidx][loc * 128:(loc + 1) * 128, :],
                                  in_=st[:])
                if loc == GS[gidx] - 1:
                    r0 = RG[gidx] * NCORES * 128
                    r1 = RG[gidx + 1] * NCORES * 128
                    nc.gpsimd.collective_compute(
                        "AllGather", OP.bypass,
                        replica_groups=[list(range(NCORES))],
                        ins=[H1g[gidx].opt()],
                        outs=[H1[r0:r1, :].opt()],
                    )

            # ---- layer-1 finalize: normalize + relu + MLP head ----
            def fin1(t, ps_o, ps_s):
                sp = work.tile([128, 4], F32, tag="sp")
                nc.vector.tensor_scalar_add(sp[:], ps_s[:, 0:4], 1e-16)
                rc = work.tile([128, 4], F32, tag="rc")
                nc.vector.reciprocal(rc[:], sp[:])
                ao = work.tile([128, HC], F32, tag="ao", bufs=2)
                nc.vector.tensor_mul(
                    ao[:].rearrange("x (h c) -> x h c", h=H),
                    ps_o[:].rearrange("x (h c) -> x h c", h=H),
                    _bcast_last(rc[:], C))
                if use_b1:
                    ab = work.tile([128, HC], F32, tag="ao", bufs=2)
                    nc.vector.tensor_add(ab[:], ao[:], b1_s[:])
                    ao2 = ab
                else:
                    ao2 = ao
                ar = work.tile([128, HC], BF16, tag="ar1", bufs=2)
                nc.vector.tensor_scalar_max(ar[:], ao2[:], 0.0)
                h2p = psfin.tile([128, FTS], F32, tag="pf")
                for kk in range(KB):
                    pt = pstr.tile([128, 128], BF16, tag="pt")
                    nc.tensor.transpose(pt[:], ar[:, kk * 128:(kk + 1) * 128],
                                        idb_s[:])
                    a1k = tp.tile([128, 128], BF16, tag="a1T32")
                    nc.scalar.activation(a1k[:], pt[:], AF.Copy)
                    nc.tensor.matmul(h2p[:], a1k[:], r2_s[kk][:],
                                     start=(kk == 0), stop=(kk == KB - 1))
                if use_l0b:
                    h2b = work.tile([128, FTS], F32, tag="h2b")
                    nc.vector.tensor_add(h2b[:], h2p[:], l0b_s[:])
                else:
                    h2b = h2p
                h2r = work.tile([128, FTS], BF16, tag="h2r")
                nc.vector.tensor_scalar_max(h2r[:], h2b[:], 0.0)
                pt2 = pstr.tile([128, 128], BF16, tag="pt")
                nc.tensor.transpose(pt2[:], h2r[:], idb_s[:])
                h2T = tp.tile([128, 128], BF16, tag="a1T32")
                nc.scalar.activation(h2T[:], pt2[:], AF.Copy)
                po = pss.tile([128, 8], F32, tag="pss")
                nc.tensor.matmul(po[:, 0:1], h2T[:], r3_s[:],
                                 start=True, stop=True)
                ob = work.tile([128, 1], F32, tag="ob")
                if use_l1b:
                    nc.vector.tensor_add(ob[:], po[:, 0:1], l1b_s[:])
                else:
                    nc.vector.tensor_copy(ob[:], po[:, 0:1])
                nc.sync.dma_start(out=outp[t * 128:(t + 1) * 128, :],
                                  in_=ob[:])

            # pead for layer 0 is pipelined into the aggregation loop:
            # before tile t's chunks run, peads are emitted for every
            # tile any super ensured during t can touch
            pead_next = [0]

            def pre0(tl):
                while pead_next[0] <= tl:
                    pead_tile(pead_next[0], adt0_s, pd0_all)
                    pead_next[0] += 1

            nc._state.push_named_scope("phaseB")
            agg_layer("l0", 80, pd0_all, fin0, pre_tile=pre0)
            nc._state.pop_named_scope("phaseB")
            nc._state.push_named_scope("phaseD")
            agg_layer("l1", TW, pd1_all, fin1)
            nc._state.pop_named_scope("phaseD")

    nc.finalize()
    return nc


def _wrap_idx(v, E_pad):
    blk = np.zeros((16, E_pad // 16), np.int16)
    ar = np.arange(E_pad)
    blk[ar % 16, ar // 16] = v.astype(np.int16)
    return np.tile(blk, (8, 1))


def kernel(x, edge_index, edge_weights,
           W0, as0, ad0, We0, ae0, b0,
           W1, as1, ad1, We1, ae1, b1,
           L0W, L0b, L1W, L1b):
    x = np.asarray(x, np.float32)
    N, F_IN = x.shape
    HC = W0.shape[0]
    H, C = np.asarray(as0).shape
    FTS = np.asarray(L0W).shape[0]

    NT = -(-N // (128 * NCORES))
    SHARD = NT * 128
    NP = SHARD * NCORES
    # allgather groups (tile counts): sized so each group's transfer keeps
    # pace with layer-0 tile production; 1-tile tail minimizes the
    # phase-boundary stall
    if NT == 20:
        GS = [4, 4, 4, 4, 2, 2]   # pair-aligned (tiles finalize in pairs)
    else:
        GS = [NT]
    RG = np.zeros(len(GS) + 1, np.int64)
    RG[1:] = np.cumsum(GS)
    g_of_t = np.repeat(np.arange(len(GS)), GS)

    # ---- edges ----
    ew_in = np.asarray(edge_weights, np.float32)
    src = np.concatenate([np.asarray(edge_index[0]), np.arange(N)])
    dst = np.concatenate([np.asarray(edge_index[1]), np.arange(N)])
    ew = np.concatenate([ew_in, np.full(N, ew_in.mean(), np.float32)])

    # ---- degree-balanced node -> (core, tile, slot) assignment ----
    # LPT-pack nodes into NTG bins of 128 by in-degree, then deal bins to
    # (tile, core) rank-major so per-tile maxima (=> K_t padding) equalize
    NTG = NP // 128
    nodes = np.arange(NP)
    deg = np.bincount(dst, minlength=NP)
    import heapq
    heap = [(0, b) for b in range(NTG)]
    heapq.heapify(heap)
    bincnt = np.zeros(NTG, np.int64)
    binsum = np.zeros(NTG, np.int64)
    bin_of_n = np.empty(NP, np.int64)
    for n in np.argsort(-deg, kind="stable"):
        while True:
            s, b = heapq.heappop(heap)
            if bincnt[b] < 128:
                break
        bin_of_n[n] = b
        binsum[b] += deg[n]
        bincnt[b] += 1
        if bincnt[b] < 128:
            heapq.heappush(heap, (int(binsum[b]), b))
    brank = np.argsort(-binsum, kind="stable")
    tile_of_bin = np.empty(NTG, np.int64)
    core_of_bin = np.empty(NTG, np.int64)
    tile_of_bin[brank] = np.arange(NTG) // NCORES
    core_of_bin[brank] = np.arange(NTG) % NCORES
    n_tile = tile_of_bin[bin_of_n]
    n_core = core_of_bin[bin_of_n]
    gt_of_n = n_core * NT + n_tile               # node -> global tile
    ord2 = np.argsort(gt_of_n, kind="stable")
    n_slot = np.empty(NP, np.int64)
    n_slot[ord2] = np.arange(NP) % 128
    node_of = np.empty(NP, np.int64)             # (gtile*128+slot) -> node
    node_of[gt_of_n * 128 + n_slot] = nodes

    # table-row permutation (group-major) so group AllGathers land contiguous
    gg = g_of_t[n_tile]
    t_of_n = (RG[gg] * NCORES * 128 + n_core * np.asarray(GS)[gg] * 128
              + (n_tile - RG[gg]) * 128 + n_slot)    # node -> table row

    order = np.argsort(gt_of_n[dst], kind="stable")
    src_s, dst_s, ew_s = src[order], dst[order], ew[order]

    tile_of = gt_of_n[dst_s]
    tcounts = np.bincount(tile_of, minlength=NTG)
    tstart = np.concatenate([[0], np.cumsum(tcounts)])

    K_t = [max(1, int(max(-(-tcounts[i * NT + t] // 128)
                          for i in range(NCORES))))
           for t in range(NT)]
    # equalize within pairs so the interleaved schedule has regular stride
    for j2 in range(0, NT - 1, 2):
        m = max(K_t[j2], K_t[j2 + 1])
        K_t[j2] = K_t[j2 + 1] = m
    NCHUNK = int(sum(K_t))
    E_pad = NCHUNK * 128
    qstart_h = np.concatenate([[0], np.cumsum(K_t)]).astype(np.int64)

    # pair-interleaved chunk schedule: tiles 2j/2j+1 alternate chunks so
    # src-sorted gathers unlock group-by-group across two tiles at once
    seq = []
    tile_slots = [[] for _ in range(NT)]
    for j2 in range(0, NT, 2):
        ta, tb = j2, min(j2 + 1, NT - 1)
        for k in range(K_t[ta]):
            tile_slots[ta].append(len(seq))
            seq.append((ta, k))
            if tb != ta and k < K_t[tb]:
                tile_slots[tb].append(len(seq))
                seq.append((tb, k))

    # ---- weight folding (host, O(weights) + O(N*F_IN*H)) ----
    as0 = np.asarray(as0, np.float32)
    ad0 = np.asarray(ad0, np.float32)
    ae0w = np.asarray(ae0, np.float32)
    as1 = np.asarray(as1, np.float32)
    ad1 = np.asarray(ad1, np.float32)
    ae1w = np.asarray(ae1, np.float32)
    W0 = np.asarray(W0, np.float32)
    W1 = np.asarray(W1, np.float32)
    We0 = np.asarray(We0, np.float32)
    We1 = np.asarray(We1, np.float32)

    k0 = (We0.reshape(H, C) * ae0w).sum(1).astype(np.float32)
    k1 = (We1.reshape(H, C) * ae1w).sum(1).astype(np.float32)

    def fold(W, a):
        blk = np.zeros((HC, H), np.float32)
        for h in range(H):
            blk[h * C:(h + 1) * C, h] = a[h]
        return (W.T @ blk).astype(np.float32)

    bf = ml_dtypes.bfloat16
    r1h = W1.T.astype(bf)
    r1a = np.concatenate([fold(W1, as1), fold(W1, ad1)], 1).astype(bf)
    r2 = np.asarray(L0W, np.float32).T.astype(bf)
    r3 = np.asarray(L1W, np.float32).T.astype(bf)
    w0h = W0.T.astype(bf)           # [64, 512]; cols h*128.. = W0_h^T

    # per-node layer-0 attention terms (tiny host matmuls)
    asrc0 = (x @ fold(W0, as0)).astype(np.float32)   # [N, 4]
    adst0 = (x @ fold(W0, ad0)).astype(np.float32)   # [N, 4]

    xbf = x.astype(bf)                               # node-order x, bf16
    adsta = np.zeros((NP, 4), np.float32)
    adsta[:N] = adst0

    b1t = np.tile(np.asarray(b1, np.float32)[None, :], (128, 1))
    l0bt = np.tile(np.asarray(L0b, np.float32)[None, :], (128, 1))
    l1bt = np.tile(np.asarray(L1b, np.float32).reshape(1, 1), (128, 1))
    ident = np.eye(128, dtype=np.float32)
    identb = np.eye(128, dtype=np.float32).astype(bf)

    in_maps = []
    srcp_all = []
    for i in range(NCORES):
        srcp = np.zeros(E_pad, np.int64)
        srcn = np.zeros(E_pad, np.int64)         # node-id src (host gather)
        dlocp = np.full(E_pad, -1, np.int64)
        dlocp_tm = np.full(E_pad, -1, np.int64)  # tile-major (oht/pead)
        ewp = np.zeros(E_pad, np.float32)
        for t in range(NT):
            gt = i * NT + t
            cnt = int(tcounts[gt])
            sl = slice(tstart[gt], tstart[gt] + cnt)
            # order tile's edges by src table row: the L1 gather then
            # walks H1 monotonically (better HBM behavior, dups adjacent)
            so = np.argsort(t_of_n[src_s[sl]], kind="stable")
            tsp = t_of_n[src_s[sl]][so]
            tsn = src_s[sl][so]
            tdl = n_slot[dst_s[sl]][so]
            tew = ew_s[sl][so]
            o_tm = qstart_h[t] * 128
            dlocp_tm[o_tm:o_tm + cnt] = tdl
            for k in range(K_t[t]):
                lo = k * 128
                if lo >= cnt:
                    break
                hi2 = min(lo + 128, cnt)
                qd = tile_slots[t][k] * 128
                srcp[qd:qd + hi2 - lo] = tsp[lo:hi2]
                srcn[qd:qd + hi2 - lo] = tsn[lo:hi2]
                dlocp[qd:qd + hi2 - lo] = tdl[lo:hi2]
                ewp[qd:qd + hi2 - lo] = tew[lo:hi2]
        srcp_all.append(srcp.copy())
        ae1p = (ewp[:, None] * k1[None, :]).reshape(
            NCHUNK, 128, 4).transpose(1, 0, 2)
        # layer-0 edge stream: [x(64) | 1 | 0*3 | asrc0+ae0 f32 bits | pad]
        xg_u16 = np.zeros((E_pad, 80), np.uint16)
        xg_u16[:, 0:64] = xbf[srcn].view(np.uint16)
        xg_u16[:, 64] = np.array(1.0, bf).view(np.uint16)
        l0a = (asrc0[srcn] + ewp[:, None] * k0[None, :]).astype(np.float32)
        xg_u16[:, 68:76] = np.ascontiguousarray(l0a).view(np.uint16)
        xgs_np = np.ascontiguousarray(
            xg_u16.reshape(NCHUNK, 128, 80).transpose(1, 0, 2)
            .reshape(128, NCHUNK * 80)).view(bf)
        dl2 = dlocp.reshape(NCHUNK, 128)
        valid = dl2 >= 0
        qs, es = np.nonzero(valid)
        f8 = ml_dtypes.float8_e4m3fn
        # ohb[e, q, d]: partition = edge-slot e
        ohb_np = np.zeros((128, NCHUNK, 128), f8)
        ohb_np[es, qs, dl2[qs, es]] = 1.0
        ohb_np = np.ascontiguousarray(ohb_np.reshape(128, NCHUNK * 128))
        # oht[d, q, e]: partition = dst-slot d; TILE-MAJOR chunk order
        dl2t = dlocp_tm.reshape(NCHUNK, 128)
        qs2, es2 = np.nonzero(dl2t >= 0)
        oht_np = np.zeros((128, NCHUNK, 128), f8)
        oht_np[dl2t[qs2, es2], qs2, es2] = 1.0
        oht_np = np.ascontiguousarray(oht_np.reshape(128, NCHUNK * 128))
        # adst0 rows for this core's dst tiles
        adt0_np = np.zeros((128, NT * 4), np.float32)
        for t in range(NT):
            rows = node_of[(i * NT + t) * 128 + np.arange(128)]
            adt0_np[:, t * 4:(t + 1) * 4] = adsta[rows]
        in_maps.append({
            "xgs": xgs_np, "r1h": r1h, "r1a": r1a, "r2": r2, "r3": r3,
            "w0h": w0h, "b1t": b1t, "l0bt": l0bt, "l1bt": l1bt,
            "ident": ident, "identb": identb,
            "srcw": _wrap_idx(srcp, E_pad),
            "ohb": ohb_np, "oht": oht_np,
            "ae1": np.ascontiguousarray(ae1p).astype(bf),
            "adt0": adt0_np.astype(bf),
        })

    hi_q = (np.stack(srcp_all).reshape(NCORES, NCHUNK, 128)
            .max(axis=(0, 2)) + 1)
    nc = _build_program(NP, F_IN, HC, H, C, NT, K_t, FTS, GS, hi_q,
                        seq, tile_slots,
                        bool(np.any(b0)), bool(np.any(b1)),
                        bool(np.any(np.asarray(L0b))),
                        bool(np.any(np.asarray(L1b))))
    res = run_bass_kernel_spmd(nc, in_maps, list(range(NCORES)))
    cat = np.concatenate([res.results[i]["out"][:, 0] for i in range(NCORES)])
    out_full = np.empty(NP, np.float32)
    out_full[node_of] = cat
    return out_full[:N].astype(np.float32)

